# revision 1
# baseline (speedup 1.0000x reference)
"""Banded local-linear layer (nn_LocalLinearLayer) on 8 trn2 NeuronCores.

out[b, o, c] = sum_p W[o, p] * xpad[b, c, p] + bias[o],  band p in [o, o+25)
xpad = edge-replicate pad of x along L (first/last 12 rows duplicated).

Strategy (v3):
  - Data-parallel over batch: 4 batches per core; banded weights replicated.
  - Output tiled in 104-row tiles: tile t = out rows [104t, 104t+104), contracts
    over xpad rows [104t, 104t+128) -> ONE K=128 matmul per tile (40 tiles).
  - Host pre-shuffles xpad into the exact SBUF layout [128, tile, b*64+c] (fp16)
    and unshuffles the output, so every DMA is fully contiguous (large
    descriptors, no strided-DMA penalty) and the device loop is uniform.
  - fp16 operands, fp32 PSUM accumulation, fp32 bias/output (~4e-4 rel err).
  - PSUM->SBUF + bias alternates ScalarE activation / VectorE tensor_scalar_add.
  - x/out staged in 4 chunks of 10 tiles for DMA/compute overlap; input DMAs on
    the Sync HWDGE ring, output DMAs on the Scalar ring.
"""

import sys

for _p in ("/opt/trn_rl_repo",):
    if _p not in sys.path:
        sys.path.insert(0, _p)

import numpy as np

import concourse.bass as bass
import concourse.tile as tile
from concourse import bacc, mybir
from concourse.bass_utils import run_bass_kernel_spmd

L = 4096
WIN = 25
PAD = (WIN - 1) // 2  # 12
PADDED = L + 2 * PAD  # 4120
B = 32
C = 64
NCORES = 8
BPC = B // NCORES  # 4
P = 128
M = P - (WIN - 1)  # 104 output rows per tile
NT = (L + M - 1) // M  # 40 tiles
M_LAST = L - (NT - 1) * M  # 40
NFREE = BPC * C  # 256
NCHUNK = 4
TPC = NT // NCHUNK  # 10

F32 = mybir.dt.float32
F16 = mybir.dt.float16


def _host_weights(W: np.ndarray, b: np.ndarray):
    o = np.arange(L)[:, None]
    p = np.arange(PADDED)[None, :]
    Wm = np.where((p >= o) & (p < o + WIN), W, 0.0).astype(np.float32)
    # wb[k, t, m] = Wm[t*104+m, t*104+k], zero-padded out of range
    wb = np.zeros((P, NT, M), np.float32)
    bias_t = np.zeros((M, NT), np.float32)
    for t in range(NT):
        mt = min(M, L - t * M)
        kt = min(P, PADDED - t * M)
        wb[:kt, t, :mt] = Wm[t * M : t * M + mt, t * M : t * M + kt].T
        bias_t[:mt, t] = b[t * M : t * M + mt]
    return wb.astype(np.float16), bias_t


def _host_x(x: np.ndarray):
    """x [B, L, C] f32 -> [P, NT, B, C] f16 in xpad-tile layout."""
    xp = np.concatenate([x[:, :PAD], x, x[:, -PAD:]], axis=1).astype(np.float16)
    xh = np.zeros((P, NT, B, C), np.float16)
    for t in range(NT):
        kt = min(P, PADDED - t * M)
        xh[:kt, t] = xp[:, t * M : t * M + kt].transpose(1, 0, 2)
    return xh


def _build_nc():
    nc = bacc.Bacc("TRN2", target_bir_lowering=False, debug=False, num_devices=NCORES)
    x_d = nc.dram_tensor("x", [P, NT, NFREE], F16, kind="ExternalInput").ap()
    wb_d = nc.dram_tensor("wb", [P, NT, M], F16, kind="ExternalInput").ap()
    bias_d = nc.dram_tensor("bias", [M, NT], F32, kind="ExternalInput").ap()
    out_d = nc.dram_tensor("out", [M, NT, NFREE], F32, kind="ExternalOutput").ap()

    with tile.TileContext(nc) as tc:
        with (
            tc.tile_pool(name="main", bufs=1) as pool,
            tc.tile_pool(name="ps", bufs=8, space=bass.MemorySpace.PSUM) as pspool,
        ):
            wb_s = pool.tile([P, NT, M], F16)
            bias_s = pool.tile([M, NT], F32)
            xch = [
                pool.tile([P, TPC, NFREE], F16, name=f"xch{c}") for c in range(NCHUNK)
            ]
            sch = [
                pool.tile([M, TPC, NFREE], F32, name=f"sch{c}") for c in range(NCHUNK)
            ]

            nc.sync.dma_start(wb_s[:], wb_d)
            nc.sync.dma_start(bias_s[:], bias_d)
            for ch in range(NCHUNK):
                nc.sync.dma_start(
                    xch[ch][:], x_d[:, ch * TPC : (ch + 1) * TPC, :]
                )

            for t in range(NT):
                c, j = t // TPC, t % TPC
                ps = pspool.tile([M, NFREE], F32)
                nc.tensor.matmul(
                    ps[:], wb_s[:, t], xch[c][:, j, :], start=True, stop=True
                )
                if t % 2 == 0:
                    nc.scalar.activation(
                        sch[c][:, j, :],
                        ps[:],
                        mybir.ActivationFunctionType.Identity,
                        bias=bias_s[:, t : t + 1],
                    )
                else:
                    nc.vector.tensor_scalar_add(
                        sch[c][:, j, :], ps[:], bias_s[:, t : t + 1]
                    )

            for ch in range(NCHUNK):
                nc.scalar.dma_start(
                    out_d[:, ch * TPC : (ch + 1) * TPC, :], sch[ch][:]
                )

    nc.compile()
    return nc


_NC = None


def _get_nc():
    global _NC
    if _NC is None:
        _NC = _build_nc()
    return _NC


def _make_in_maps(x, W, b):
    wb, bias_t = _host_weights(
        np.asarray(W, dtype=np.float32), np.asarray(b, dtype=np.float32)
    )
    xh = _host_x(np.asarray(x, dtype=np.float32))
    return [
        {
            "x": np.ascontiguousarray(
                xh[:, :, c * BPC : (c + 1) * BPC, :]
            ).reshape(P, NT, NFREE),
            "wb": wb,
            "bias": bias_t,
        }
        for c in range(NCORES)
    ]


def _gather(results):
    oh = np.concatenate(
        [r["out"].reshape(M, NT, BPC, C) for r in results], axis=2
    )  # [104, NT, B, C]
    out = np.empty((B, L, C), np.float32)
    for t in range(NT):
        mt = min(M, L - t * M)
        out[:, t * M : t * M + mt] = oh[:mt, t].transpose(1, 0, 2)
    return out


def kernel(x: np.ndarray, W: np.ndarray, b: np.ndarray) -> np.ndarray:
    nc = _get_nc()
    res = run_bass_kernel_spmd(nc, _make_in_maps(x, W, b), list(range(NCORES)))
    return _gather(res.results)


if __name__ == "__main__":
    rng = np.random.default_rng(0)
    x = rng.standard_normal((B, L, C), dtype=np.float32)
    W = rng.standard_normal((L, PADDED), dtype=np.float32) * 0.02
    b = rng.standard_normal((L,), dtype=np.float32) * 0.02
    print(kernel(x, W, b).shape)



# revision 2
# speedup vs baseline: 1.2756x; 1.2756x over previous
"""Banded local-linear layer (nn_LocalLinearLayer) on 8 trn2 NeuronCores.

out[b, o, c] = sum_p W[o, p] * xpad[b, c, p] + bias[o],  band p in [o, o+25)
xpad = concat(x[:12], x, x[-12:]) along L (first/last 12 rows duplicated).

Strategy (v4, tensor-parallel over L):
  - Each core owns 512 output rows (L/8); the free dim is all B*C = 2048 cols.
    This cuts replicated-weight HBM traffic 7x vs batch-parallel.
  - xpad viewed as 128-row blocks [128, 33, B*C]: out tile t (128 rows)
    contracts over xpad rows [128t, 128t+152) = block t (K=128 matmul)
    + first 24 rows of block t+1 (K=24 matmul) accumulating in PSUM.
    Blocks are partition-aligned -> x is DMAd with ZERO duplication.
  - fp16 operands and fp16 output (fp32 PSUM accumulation, fp32 bias):
    halves output HBM bytes vs f32 and uses all 16 SDMA engines (128 parts).
  - Out-DMA per 128-row tile issued as soon as its 4 PSUM chunks are done,
    overlapping the remaining input DMAs and compute.
  - PSUM->SBUF + bias alternates ScalarE activation / VectorE tensor_scalar.
"""

import sys

for _p in ("/opt/trn_rl_repo",):
    if _p not in sys.path:
        sys.path.insert(0, _p)

import numpy as np

import concourse.bass as bass
import concourse.tile as tile
from concourse import bacc, mybir
from concourse.bass_utils import run_bass_kernel_spmd

L = 4096
WIN = 25
PAD = (WIN - 1) // 2  # 12
PADDED = L + 2 * PAD  # 4120
B = 32
C = 64
NCORES = 8
P = 128
RPC = L // NCORES  # 512 output rows per core
NT = RPC // P  # 4 tiles of 128 rows per core
HALO = WIN - 1  # 24
NF = B * C  # 2048 free columns
NCH = 4
CHUNK = NF // NCH  # 512 (one PSUM bank of fp32)
NBLK = (PADDED + P - 1) // P  # 33 blocks of xpad rows

F32 = mybir.dt.float32
F16 = mybir.dt.float16


def _host_weights(W: np.ndarray, b: np.ndarray):
    """Per-core banded stationary operands.

    w1[c][k, t, m] = Wm[base+m, base+k]        (k in [0,128))
    w2[c][k, t, m] = Wm[base+m, base+128+k]    (k in [0,24))
    bias[c][m, t]  = b[base+m],   base = 512c + 128t
    """
    o = np.arange(L)[:, None]
    p = np.arange(PADDED)[None, :]
    Wm = np.where((p >= o) & (p < o + WIN), W, 0.0).astype(np.float32)
    w1 = np.zeros((NCORES, P, NT, P), np.float16)
    w2 = np.zeros((NCORES, HALO, NT, P), np.float16)
    bias = np.zeros((NCORES, P, NT), np.float32)
    for c in range(NCORES):
        for t in range(NT):
            base = RPC * c + P * t
            w1[c, :, t, :] = Wm[base : base + P, base : base + P].T
            w2[c, :, t, :] = Wm[base : base + P, base + P : base + P + HALO].T
            bias[c, :, t] = b[base : base + P]
    return w1, w2, bias


def _host_x(x: np.ndarray):
    """x [B, L, C] f32 -> xh [P, NBLK, B*C] f16, xh[p, blk, f] = xpad[b, 128*blk+p, c]."""
    xp = np.concatenate([x[:, :PAD], x, x[:, -PAD:]], axis=1).astype(np.float16)
    xpb = np.zeros((B, NBLK * P, C), np.float16)
    xpb[:, :PADDED] = xp
    # [B, NBLK, P, C] -> [P, NBLK, B, C]
    xh = xpb.reshape(B, NBLK, P, C).transpose(2, 1, 0, 3).reshape(P, NBLK, NF)
    return xh


def _build_nc():
    nc = bacc.Bacc("TRN2", target_bir_lowering=False, debug=False, num_devices=NCORES)
    xm_d = nc.dram_tensor("xm", [P, NT, NF], F16, kind="ExternalInput").ap()
    xe_d = nc.dram_tensor("xe", [HALO, NF], F16, kind="ExternalInput").ap()
    w1_d = nc.dram_tensor("w1", [P, NT, P], F16, kind="ExternalInput").ap()
    w2_d = nc.dram_tensor("w2", [HALO, NT, P], F16, kind="ExternalInput").ap()
    bias_d = nc.dram_tensor("bias", [P, NT], F32, kind="ExternalInput").ap()
    out_d = nc.dram_tensor("out", [P, NT, NF], F16, kind="ExternalOutput").ap()

    with tile.TileContext(nc) as tc:
        with (
            tc.tile_pool(name="main", bufs=1) as pool,
            tc.tile_pool(name="ps", bufs=8, space=bass.MemorySpace.PSUM) as pspool,
        ):
            w1_s = pool.tile([P, NT, P], F16)
            w2_s = pool.tile([HALO, NT, P], F16)
            bias_s = pool.tile([P, NT], F32)
            xs = [pool.tile([P, NF], F16, name=f"x{t}") for t in range(NT)]
            xs.append(pool.tile([HALO, NF], F16, name="xe"))
            obs = [pool.tile([P, NF], F16, name=f"o{t}") for t in range(NT)]

            nc.scalar.dma_start(w1_s[:], w1_d)
            nc.scalar.dma_start(w2_s[:], w2_d)
            nc.scalar.dma_start(bias_s[:], bias_d)
            for t in range(NT):
                nc.sync.dma_start(xs[t][:], xm_d[:, t, :])
            nc.sync.dma_start(xs[NT][:], xe_d)

            for t in range(NT):
                for j in range(NCH):
                    sl = slice(j * CHUNK, (j + 1) * CHUNK)
                    ps = pspool.tile([P, CHUNK], F32)
                    nc.tensor.matmul(
                        ps[:], w1_s[:, t], xs[t][:, sl], start=True, stop=False
                    )
                    nc.tensor.matmul(
                        ps[:], w2_s[:, t], xs[t + 1][0:HALO, sl], start=False, stop=True
                    )
                    if j % 2 == 0:
                        nc.scalar.activation(
                            obs[t][:, sl],
                            ps[:],
                            mybir.ActivationFunctionType.Identity,
                            bias=bias_s[:, t : t + 1],
                        )
                    else:
                        nc.vector.tensor_scalar_add(
                            obs[t][:, sl], ps[:], bias_s[:, t : t + 1]
                        )
                nc.scalar.dma_start(out_d[:, t, :], obs[t][:])

    nc.compile()
    return nc


_NC = None


def _get_nc():
    global _NC
    if _NC is None:
        _NC = _build_nc()
    return _NC


def _make_in_maps(x, W, b):
    w1, w2, bias = _host_weights(
        np.asarray(W, dtype=np.float32), np.asarray(b, dtype=np.float32)
    )
    xh = _host_x(np.asarray(x, dtype=np.float32))
    maps = []
    for c in range(NCORES):
        maps.append(
            {
                "xm": np.ascontiguousarray(xh[:, NT * c : NT * c + NT, :]),
                "xe": np.ascontiguousarray(xh[:HALO, NT * c + NT, :]),
                "w1": w1[c],
                "w2": w2[c],
                "bias": bias[c],
            }
        )
    return maps


def _gather(results):
    out = np.empty((B, L, C), np.float32)
    for c in range(NCORES):
        oh = results[c]["out"].astype(np.float32)  # [P, NT, NF]
        # [P, NT, B, C] -> [B, NT, P, C] -> [B, 512, C]
        o4 = oh.reshape(P, NT, B, C).transpose(2, 1, 0, 3).reshape(B, RPC, C)
        out[:, RPC * c : RPC * (c + 1)] = o4
    return out


def kernel(x: np.ndarray, W: np.ndarray, b: np.ndarray) -> np.ndarray:
    nc = _get_nc()
    res = run_bass_kernel_spmd(nc, _make_in_maps(x, W, b), list(range(NCORES)))
    return _gather(res.results)


if __name__ == "__main__":
    rng = np.random.default_rng(0)
    x = rng.standard_normal((B, L, C), dtype=np.float32)
    W = rng.standard_normal((L, PADDED), dtype=np.float32) * 0.02
    b = rng.standard_normal((L,), dtype=np.float32) * 0.02
    print(kernel(x, W, b).shape)


# revision 3
# speedup vs baseline: 1.4088x; 1.1044x over previous
"""Banded local-linear layer (nn_LocalLinearLayer) on 8 trn2 NeuronCores.

out[b, o, c] = sum_p W[o, p] * xpad[b, c, p] + bias[o],  band p in [o, o+25)
xpad = concat(x[:12], x, x[-12:]) along L (first/last 12 rows duplicated).

Strategy (v5, tensor-parallel over L, single matmul per tile):
  - Each core owns 512 output rows (L/8); free dim = all B*C = 2048 cols.
    Weights are L-sharded so replicated-weight HBM traffic stays tiny.
  - Output tiled in M=104-row tiles (5 per core: 4x104 + 1x96); tile t
    contracts over xpad rows [104t, 104t+128) -> ONE K=128 matmul per
    512-col PSUM chunk (PE streams = 20x512 cols/core, the minimum for
    a 1.2GHz-throttled PE; a K-split variant costs 2x PE time).
  - Host pre-shuffles x into partition-aligned tile layout (23% duplicate
    HBM bytes, traded for halved PE stream time).
  - fp16 operands + fp16 output (fp32 PSUM accum, fp32 bias).
  - PSUM->SBUF + bias alternates ScalarE activation / VectorE tensor_scalar;
    x loads on the Sync HWDGE ring, weights on the Scalar ring, output
    stores on the GpSimd SWDGE ring so no engine queue is oversubscribed.
  - Per-tile out-DMA issued as soon as the tile's 4 chunks are copied,
    overlapping remaining input DMA + compute.
"""

import sys

for _p in ("/opt/trn_rl_repo",):
    if _p not in sys.path:
        sys.path.insert(0, _p)

import numpy as np

import concourse.bass as bass
import concourse.tile as tile
from concourse import bacc, mybir
from concourse.bass_utils import run_bass_kernel_spmd

L = 4096
WIN = 25
PAD = (WIN - 1) // 2  # 12
PADDED = L + 2 * PAD  # 4120
B = 32
C = 64
NCORES = 8
P = 128
RPC = L // NCORES  # 512 output rows per core
M = 104  # output rows per tile (K = M + WIN - 1 = 128)
NT = (RPC + M - 1) // M  # 5 tiles per core
M_LAST = RPC - (NT - 1) * M  # 96
K_LAST = M_LAST + WIN - 1  # 120
NF = B * C  # 2048 free columns
NCH = 4
CHUNK = NF // NCH  # 512 (one PSUM bank of fp32)

F32 = mybir.dt.float32
F16 = mybir.dt.float16


def _host_weights(W: np.ndarray, b: np.ndarray):
    """w1[c][k, t, m] = Wm[base+m, base+k], bias[c][m, t] = b[base+m],
    base = 512c + 104t."""
    o = np.arange(L)[:, None]
    p = np.arange(PADDED)[None, :]
    Wm = np.where((p >= o) & (p < o + WIN), W, 0.0).astype(np.float32)
    w1 = np.zeros((NCORES, P, NT, M), np.float16)
    bias = np.zeros((NCORES, M, NT), np.float32)
    for c in range(NCORES):
        for t in range(NT):
            base = RPC * c + M * t
            mt = M if t < NT - 1 else M_LAST
            kt = P if t < NT - 1 else K_LAST
            w1[c, :kt, t, :mt] = Wm[base : base + mt, base : base + kt].T
            bias[c, :mt, t] = b[base : base + mt]
    return w1, bias


def _host_x(x: np.ndarray):
    """x [B, L, C] f32 -> per-core [P, NT, NF] f16 tile layout,
    xc[c][k, t, f] = xpad[b, 512c + 104t + k, ch]  (f = 64b + ch)."""
    xp = np.concatenate([x[:, :PAD], x, x[:, -PAD:]], axis=1).astype(np.float16)
    xcs = []
    for c in range(NCORES):
        xc = np.zeros((P, NT, NF), np.float16)
        for t in range(NT):
            base = RPC * c + M * t
            kt = P if t < NT - 1 else K_LAST
            xc[:kt, t] = xp[:, base : base + kt].transpose(1, 0, 2).reshape(kt, NF)
        xcs.append(xc)
    return xcs


def _build_nc():
    nc = bacc.Bacc("TRN2", target_bir_lowering=False, debug=False, num_devices=NCORES)
    xm_d = nc.dram_tensor("xm", [P, NT, NF], F16, kind="ExternalInput").ap()
    w1_d = nc.dram_tensor("w1", [P, NT, M], F16, kind="ExternalInput").ap()
    bias_d = nc.dram_tensor("bias", [M, NT], F32, kind="ExternalInput").ap()
    out_d = nc.dram_tensor("out", [M, NT, NF], F16, kind="ExternalOutput").ap()

    with tile.TileContext(nc) as tc:
        with (
            tc.tile_pool(name="main", bufs=1) as pool,
            tc.tile_pool(name="ps", bufs=8, space=bass.MemorySpace.PSUM) as pspool,
        ):
            w1_s = pool.tile([P, NT, M], F16)
            bias_s = pool.tile([M, NT], F32)
            xs = [pool.tile([P, NF], F16, name=f"x{t}") for t in range(NT)]
            obs = [pool.tile([M, NF], F16, name=f"o{t}") for t in range(NT)]

            nc.scalar.dma_start(w1_s[:], w1_d)
            nc.scalar.dma_start(bias_s[:], bias_d)
            for t in range(NT):
                nc.sync.dma_start(xs[t][:], xm_d[:, t, :])

            for t in range(NT):
                mt = M if t < NT - 1 else M_LAST
                kt = P if t < NT - 1 else K_LAST
                for j in range(NCH):
                    sl = slice(j * CHUNK, (j + 1) * CHUNK)
                    ps = pspool.tile([M, CHUNK], F32)
                    nc.tensor.matmul(
                        ps[:mt],
                        w1_s[:kt, t, :mt],
                        xs[t][:kt, sl],
                        start=True,
                        stop=True,
                    )
                    if j % 2 == 0:
                        nc.scalar.activation(
                            obs[t][:mt, sl],
                            ps[:mt],
                            mybir.ActivationFunctionType.Identity,
                            bias=bias_s[:mt, t : t + 1],
                        )
                    else:
                        nc.vector.tensor_scalar_add(
                            obs[t][:mt, sl], ps[:mt], bias_s[:mt, t : t + 1]
                        )
                nc.gpsimd.dma_start(out_d[:mt, t, :], obs[t][:mt, :])

    nc.compile()
    return nc


_NC = None


def _get_nc():
    global _NC
    if _NC is None:
        _NC = _build_nc()
    return _NC


def _make_in_maps(x, W, b):
    w1, bias = _host_weights(
        np.asarray(W, dtype=np.float32), np.asarray(b, dtype=np.float32)
    )
    xcs = _host_x(np.asarray(x, dtype=np.float32))
    return [
        {"xm": xcs[c], "w1": w1[c], "bias": bias[c]}
        for c in range(NCORES)
    ]


def _gather(results):
    out = np.empty((B, L, C), np.float32)
    for c in range(NCORES):
        oh = results[c]["out"].astype(np.float32)  # [M, NT, NF]
        for t in range(NT):
            base = RPC * c + M * t
            mt = M if t < NT - 1 else M_LAST
            # [mt, B, C] -> [B, mt, C]
            out[:, base : base + mt] = (
                oh[:mt, t].reshape(mt, B, C).transpose(1, 0, 2)
            )
    return out


def kernel(x: np.ndarray, W: np.ndarray, b: np.ndarray) -> np.ndarray:
    nc = _get_nc()
    res = run_bass_kernel_spmd(nc, _make_in_maps(x, W, b), list(range(NCORES)))
    return _gather(res.results)


if __name__ == "__main__":
    rng = np.random.default_rng(0)
    x = rng.standard_normal((B, L, C), dtype=np.float32)
    W = rng.standard_normal((L, PADDED), dtype=np.float32) * 0.02
    b = rng.standard_normal((L,), dtype=np.float32) * 0.02
    print(kernel(x, W, b).shape)


# revision 5
# speedup vs baseline: 1.4497x; 1.0290x over previous
"""Banded local-linear layer (nn_LocalLinearLayer) on 8 trn2 NeuronCores.

out[b, o, c] = sum_p W[o, p] * xpad[b, c, p] + bias[o],  band p in [o, o+25)
xpad = concat(x[:12], x, x[-12:]) along L (first/last 12 rows duplicated).

Strategy (v5, tensor-parallel over L, single matmul per tile):
  - Each core owns 512 output rows (L/8); free dim = all B*C = 2048 cols.
    Weights are L-sharded so replicated-weight HBM traffic stays tiny.
  - Output tiled in M=104-row tiles (5 per core: 4x104 + 1x96); tile t
    contracts over xpad rows [104t, 104t+128) -> ONE K=128 matmul per
    512-col PSUM chunk (PE streams = 20x512 cols/core, the minimum for
    a 1.2GHz-throttled PE; a K-split variant costs 2x PE time).
  - Host pre-shuffles x into partition-aligned tile layout (23% duplicate
    HBM bytes, traded for halved PE stream time).
  - fp16 operands + fp16 output (fp32 PSUM accum, fp32 bias).
  - PSUM->SBUF + bias alternates ScalarE activation / VectorE tensor_scalar;
    x loads on the Sync HWDGE ring, weights on the Scalar ring, output
    stores on the GpSimd SWDGE ring so no engine queue is oversubscribed.
  - Per-tile out-DMA issued as soon as the tile's 4 chunks are copied,
    overlapping remaining input DMA + compute.
"""

import sys

for _p in ("/opt/trn_rl_repo",):
    if _p not in sys.path:
        sys.path.insert(0, _p)

import numpy as np

import concourse.bass as bass
import concourse.tile as tile
from concourse import bacc, mybir
from concourse.bass_utils import run_bass_kernel_spmd

L = 4096
WIN = 25
PAD = (WIN - 1) // 2  # 12
PADDED = L + 2 * PAD  # 4120
B = 32
C = 64
NCORES = 8
P = 128
RPC = L // NCORES  # 512 output rows per core
M = 104  # output rows per tile (K = M + WIN - 1 = 128)
NT = (RPC + M - 1) // M  # 5 tiles per core
M_LAST = RPC - (NT - 1) * M  # 96
K_LAST = M_LAST + WIN - 1  # 120
NF = B * C  # 2048 free columns
NCH = 4
CHUNK = NF // NCH  # 512 (one PSUM bank of fp32)

F32 = mybir.dt.float32
F16 = mybir.dt.float16


def _host_weights(W: np.ndarray, b: np.ndarray):
    """w1[c][k, t, m] = Wm[base+m, base+k], bias[c][m, t] = b[base+m],
    base = 512c + 104t."""
    o = np.arange(L)[:, None]
    p = np.arange(PADDED)[None, :]
    Wm = np.where((p >= o) & (p < o + WIN), W, 0.0).astype(np.float32)
    w1 = np.zeros((NCORES, P, NT, M), np.float16)
    bias = np.zeros((NCORES, M, NT), np.float32)
    for c in range(NCORES):
        for t in range(NT):
            base = RPC * c + M * t
            mt = M if t < NT - 1 else M_LAST
            kt = P if t < NT - 1 else K_LAST
            w1[c, :kt, t, :mt] = Wm[base : base + mt, base : base + kt].T
            bias[c, :mt, t] = b[base : base + mt]
    return w1, bias


def _host_x(x: np.ndarray):
    """x [B, L, C] f32 -> per-core [P, NT, NF] f16 tile layout,
    xc[c][k, t, f] = xpad[b, 512c + 104t + k, ch]  (f = 64b + ch)."""
    xp = np.concatenate([x[:, :PAD], x, x[:, -PAD:]], axis=1).astype(np.float16)
    xcs = []
    for c in range(NCORES):
        xc = np.zeros((P, NT, NF), np.float16)
        for t in range(NT):
            base = RPC * c + M * t
            kt = P if t < NT - 1 else K_LAST
            xc[:kt, t] = xp[:, base : base + kt].transpose(1, 0, 2).reshape(kt, NF)
        xcs.append(xc)
    return xcs


def _build_nc():
    nc = bacc.Bacc("TRN2", target_bir_lowering=False, debug=False, num_devices=NCORES)
    xm_d = nc.dram_tensor("xm", [P, NT, NF], F16, kind="ExternalInput").ap()
    w1_d = nc.dram_tensor("w1", [P, NT, M], F16, kind="ExternalInput").ap()
    bias_d = nc.dram_tensor("bias", [M, NT], F32, kind="ExternalInput").ap()
    out_d = nc.dram_tensor("out", [M, NT, NF], F16, kind="ExternalOutput").ap()

    with tile.TileContext(nc) as tc:
        with (
            tc.tile_pool(name="main", bufs=1) as pool,
            tc.tile_pool(name="ps", bufs=8, space=bass.MemorySpace.PSUM) as pspool,
        ):
            w1_s = pool.tile([P, NT, M], F16)
            bias_s = pool.tile([M, NT], F32)
            xs = [pool.tile([P, NF], F16, name=f"x{t}") for t in range(NT)]
            obs = [pool.tile([M, NF], F16, name=f"o{t}") for t in range(NT)]

            nc.sync.dma_start(w1_s[:], w1_d)
            nc.sync.dma_start(bias_s[:], bias_d)
            for t in range(NT):
                nc.sync.dma_start(xs[t][:], xm_d[:, t, :])

            for t in range(NT):
                mt = M if t < NT - 1 else M_LAST
                kt = P if t < NT - 1 else K_LAST
                for j in range(NCH):
                    sl = slice(j * CHUNK, (j + 1) * CHUNK)
                    ps = pspool.tile([M, CHUNK], F32)
                    nc.tensor.matmul(
                        ps[:mt],
                        w1_s[:kt, t, :mt],
                        xs[t][:kt, sl],
                        start=True,
                        stop=True,
                    )
                    if j % 2 == 0:
                        nc.scalar.activation(
                            obs[t][:mt, sl],
                            ps[:mt],
                            mybir.ActivationFunctionType.Identity,
                            bias=bias_s[:mt, t : t + 1],
                        )
                    else:
                        nc.vector.tensor_scalar_add(
                            obs[t][:mt, sl], ps[:mt], bias_s[:mt, t : t + 1]
                        )
                nc.sync.dma_start(out_d[:mt, t, :], obs[t][:mt, :])

    nc.compile()
    return nc


_NC = None


def _get_nc():
    global _NC
    if _NC is None:
        _NC = _build_nc()
    return _NC


def _make_in_maps(x, W, b):
    w1, bias = _host_weights(
        np.asarray(W, dtype=np.float32), np.asarray(b, dtype=np.float32)
    )
    xcs = _host_x(np.asarray(x, dtype=np.float32))
    return [
        {"xm": xcs[c], "w1": w1[c], "bias": bias[c]}
        for c in range(NCORES)
    ]


def _gather(results):
    out = np.empty((B, L, C), np.float32)
    for c in range(NCORES):
        oh = results[c]["out"].astype(np.float32)  # [M, NT, NF]
        for t in range(NT):
            base = RPC * c + M * t
            mt = M if t < NT - 1 else M_LAST
            # [mt, B, C] -> [B, mt, C]
            out[:, base : base + mt] = (
                oh[:mt, t].reshape(mt, B, C).transpose(1, 0, 2)
            )
    return out


def kernel(x: np.ndarray, W: np.ndarray, b: np.ndarray) -> np.ndarray:
    nc = _get_nc()
    res = run_bass_kernel_spmd(nc, _make_in_maps(x, W, b), list(range(NCORES)))
    return _gather(res.results)


if __name__ == "__main__":
    rng = np.random.default_rng(0)
    x = rng.standard_normal((B, L, C), dtype=np.float32)
    W = rng.standard_normal((L, PADDED), dtype=np.float32) * 0.02
    b = rng.standard_normal((L,), dtype=np.float32) * 0.02
    print(kernel(x, W, b).shape)


# revision 6
# speedup vs baseline: 1.4964x; 1.0322x over previous
"""Banded local-linear layer (nn_LocalLinearLayer) on 8 trn2 NeuronCores.

out[b, o, c] = sum_p W[o, p] * xpad[b, c, p] + bias[o],  band p in [o, o+25)
xpad = concat(x[:12], x, x[-12:]) along L (first/last 12 rows duplicated).

Strategy (v5, tensor-parallel over L, single matmul per tile):
  - Each core owns 512 output rows (L/8); free dim = all B*C = 2048 cols.
    Weights are L-sharded so replicated-weight HBM traffic stays tiny.
  - Output tiled in M=104-row tiles (5 per core: 4x104 + 1x96); tile t
    contracts over xpad rows [104t, 104t+128) -> ONE K=128 matmul per
    512-col PSUM chunk (PE streams = 20x512 cols/core, the minimum for
    a 1.2GHz-throttled PE; a K-split variant costs 2x PE time).
  - Host pre-shuffles x into partition-aligned tile layout (23% duplicate
    HBM bytes, traded for halved PE stream time).
  - fp16 operands + fp16 output (fp32 PSUM accum, fp32 bias).
  - PSUM->SBUF + bias alternates ScalarE activation / VectorE tensor_scalar;
    x loads on the Sync HWDGE ring, weights on the Scalar ring, output
    stores on the GpSimd SWDGE ring so no engine queue is oversubscribed.
  - Per-tile out-DMA issued as soon as the tile's 4 chunks are copied,
    overlapping remaining input DMA + compute.
"""

import sys

for _p in ("/opt/trn_rl_repo",):
    if _p not in sys.path:
        sys.path.insert(0, _p)

import numpy as np

import concourse.bass as bass
import concourse.tile as tile
from concourse import bacc, mybir
from concourse.bass_utils import run_bass_kernel_spmd

L = 4096
WIN = 25
PAD = (WIN - 1) // 2  # 12
PADDED = L + 2 * PAD  # 4120
B = 32
C = 64
NCORES = 8
P = 128
RPC = L // NCORES  # 512 output rows per core
M = 104  # output rows per tile (K = M + WIN - 1 = 128)
NT = (RPC + M - 1) // M  # 5 tiles per core
M_LAST = RPC - (NT - 1) * M  # 96
K_LAST = M_LAST + WIN - 1  # 120
NF = B * C  # 2048 free columns
NCH = 4
CHUNK = NF // NCH  # 512 (one PSUM bank of fp32)

F32 = mybir.dt.float32
F16 = mybir.dt.float16


def _host_weights(W: np.ndarray, b: np.ndarray):
    """w1[c][k, t, m] = Wm[base+m, base+k], bias[c][m, t] = b[base+m],
    base = 512c + 104t."""
    o = np.arange(L)[:, None]
    p = np.arange(PADDED)[None, :]
    Wm = np.where((p >= o) & (p < o + WIN), W, 0.0).astype(np.float32)
    w1 = np.zeros((NCORES, P, NT, M), np.float16)
    bias = np.zeros((NCORES, M, NT), np.float32)
    for c in range(NCORES):
        for t in range(NT):
            base = RPC * c + M * t
            mt = M if t < NT - 1 else M_LAST
            kt = P if t < NT - 1 else K_LAST
            w1[c, :kt, t, :mt] = Wm[base : base + mt, base : base + kt].T
            bias[c, :mt, t] = b[base : base + mt]
    return w1, bias


def _host_x(x: np.ndarray):
    """x [B, L, C] f32 -> per-core [P, NT, NF] f16 tile layout,
    xc[c][k, t, f] = xpad[b, 512c + 104t + k, ch]  (f = 64b + ch)."""
    xp = np.concatenate([x[:, :PAD], x, x[:, -PAD:]], axis=1).astype(np.float16)
    xcs = []
    for c in range(NCORES):
        xc = np.zeros((P, NT, NF), np.float16)
        for t in range(NT):
            base = RPC * c + M * t
            kt = P if t < NT - 1 else K_LAST
            xc[:kt, t] = xp[:, base : base + kt].transpose(1, 0, 2).reshape(kt, NF)
        xcs.append(xc)
    return xcs


def _build_nc():
    nc = bacc.Bacc("TRN2", target_bir_lowering=False, debug=False, num_devices=NCORES)
    xm_d = nc.dram_tensor("xm", [P, NT, NF], F16, kind="ExternalInput").ap()
    w1_d = nc.dram_tensor("w1", [P, NT, M], F16, kind="ExternalInput").ap()
    bias_d = nc.dram_tensor("bias", [M, NT], F32, kind="ExternalInput").ap()
    out_d = nc.dram_tensor("out", [M, NT, NF], F16, kind="ExternalOutput").ap()

    with tile.TileContext(nc) as tc:
        with (
            tc.tile_pool(name="main", bufs=1) as pool,
            tc.tile_pool(name="ps", bufs=8, space=bass.MemorySpace.PSUM) as pspool,
        ):
            w1_s = pool.tile([P, NT, M], F16)
            bias_s = pool.tile([M, NT], F32)
            xs = [pool.tile([P, NF], F16, name=f"x{t}") for t in range(NT)]
            obs = [pool.tile([M, NF], F16, name=f"o{t}") for t in range(NT)]

            nc.sync.dma_start(w1_s[:], w1_d)
            nc.scalar.dma_start(bias_s[:], bias_d)
            for t in range(NT):
                nc.sync.dma_start(xs[t][:], xm_d[:, t, :])

            for t in range(NT):
                mt = M if t < NT - 1 else M_LAST
                kt = P if t < NT - 1 else K_LAST
                for j in range(NCH):
                    sl = slice(j * CHUNK, (j + 1) * CHUNK)
                    ps = pspool.tile([M, CHUNK], F32)
                    nc.tensor.matmul(
                        ps[:mt],
                        w1_s[:kt, t, :mt],
                        xs[t][:kt, sl],
                        start=True,
                        stop=True,
                    )
                    # ACT also issues the out DMAs, so give it 2 of 4
                    # chunks per tile and DVE the other 2... ACT 8 : DVE 12
                    if (t * NCH + j) % 5 < 2:
                        nc.scalar.activation(
                            obs[t][:mt, sl],
                            ps[:mt],
                            mybir.ActivationFunctionType.Identity,
                            bias=bias_s[:mt, t : t + 1],
                        )
                    else:
                        nc.vector.tensor_scalar_add(
                            obs[t][:mt, sl], ps[:mt], bias_s[:mt, t : t + 1]
                        )
                nc.scalar.dma_start(out_d[:mt, t, :], obs[t][:mt, :])

    nc.compile()
    return nc


_NC = None


def _get_nc():
    global _NC
    if _NC is None:
        _NC = _build_nc()
    return _NC


def _make_in_maps(x, W, b):
    w1, bias = _host_weights(
        np.asarray(W, dtype=np.float32), np.asarray(b, dtype=np.float32)
    )
    xcs = _host_x(np.asarray(x, dtype=np.float32))
    return [
        {"xm": xcs[c], "w1": w1[c], "bias": bias[c]}
        for c in range(NCORES)
    ]


def _gather(results):
    out = np.empty((B, L, C), np.float32)
    for c in range(NCORES):
        oh = results[c]["out"].astype(np.float32)  # [M, NT, NF]
        for t in range(NT):
            base = RPC * c + M * t
            mt = M if t < NT - 1 else M_LAST
            # [mt, B, C] -> [B, mt, C]
            out[:, base : base + mt] = (
                oh[:mt, t].reshape(mt, B, C).transpose(1, 0, 2)
            )
    return out


def kernel(x: np.ndarray, W: np.ndarray, b: np.ndarray) -> np.ndarray:
    nc = _get_nc()
    res = run_bass_kernel_spmd(nc, _make_in_maps(x, W, b), list(range(NCORES)))
    return _gather(res.results)


if __name__ == "__main__":
    rng = np.random.default_rng(0)
    x = rng.standard_normal((B, L, C), dtype=np.float32)
    W = rng.standard_normal((L, PADDED), dtype=np.float32) * 0.02
    b = rng.standard_normal((L,), dtype=np.float32) * 0.02
    print(kernel(x, W, b).shape)
